# revision 5
# baseline (speedup 1.0000x reference)
"""LinearSpline activation kernel for Trainium2 (8 NeuronCores, SPMD), v7.

Math: as a function of t = x*scale/grid the reference is EXACTLY a PWL with
49 slope-change knots at integers j=-24..24 plus linear extrapolation:
  f(t) = A + B*t + sum_j u_j * max(t, j),   u_j = second differences of the
coefficient table (all constants folded per channel on the host, incl. the
1/scale output factor).  The max-basis needs NO shifted input streams: knot
positions are channel-independent compile-time immediates, so a custom DVE
op (MAXPAIR: acc + C0*max(t,j) + C1*max(t,j+1), j=imm2, j+1 hoisted) handles
2 knots per instruction reading only the shared t tile + the accumulator;
MAXPAIR0 (no Src1) seeds the chain with the first two knots.
The remaining N_P knots ride the ACT engine: prelu(beta*(t-j); a) with
per-channel [P,1] scale/bias/alpha and beta*(1-a) = u_j gives a SIGNED kink
u_j*relu(t-j) plus an affine part folded into A,B on the host — one ACT op
per knot, any sign of u.  GPSIMD joins those leaves with tensor_tensor adds
into a second partial seeded from A + B*t.  The two partials are merged by
the OUTPUT DMAs: a plain store of the GPSIMD partial (sync engine) followed
by an accumulating (cce add) store of the DVE partial (Pool-issued, two
units later), so no compute engine spends an op on the merge.
Layout: data-parallel over batch (4 per core), partition p = n2*64 + channel.
"""

import numpy as np

# ---------------- problem constants (hardcoded; kernel must be standalone) --
N_BATCH, N_CH, H, W = 32, 64, 128, 128
NCORES = 8
SIZE = 51
GRID = np.float64(2.0 * 4.0 / (SIZE - 1))
FREE = H * W                     # 16384 free elems per (group, partition)
F = 2048                         # free-dim chunk per unit
GROUPS = 2                       # batch pairs per core
UNITS = GROUPS * (FREE // F)     # 16
N_KNOT = 49                      # slope-change knots j = -24..24
N_PT = 11                        # prelu-leaf knots total (ACT-produced)
N_P = 9                          # of those, joined on GPSIMD (tt add)
N_C = N_PT - N_P                 # of those, merged by accumulating DMAs
P_KS = list(range(N_PT))                     # k=0..10  -> j=-24..-14
DVE_KS = list(range(N_PT, N_KNOT))           # 38 knots -> 19 ops
N_DVE = len(DVE_KS) // 2                     # 19 (first is MAXPAIR0)
N_ACT_OPS = 2 + N_PT             # t, acc0, prelu leaves
RING = 9                         # gp-leaf ring depth (one slot per join)

_f32, _f64 = np.float32, np.float64
_built = {}


def _host_params(coeff, scal):
    """coeff [3264], scal [64] -> per-channel param table [64, 85] f32.
    cols: 0 alpha, 1 init_scale (B*alpha), 2 init_bias (A), 3+k u_k,
    52+i/63+i/74+i prelu scale/bias/alpha for leaf knot i."""
    C = coeff.reshape(N_CH, SIZE).astype(_f64)
    d = C[:, 1:] - C[:, :-1]                      # [64, 50]
    u = d[:, 1:] - d[:, :-1]                      # [64, 49]
    js = np.arange(-24, 25, dtype=_f64)
    s = scal.astype(_f64)
    alpha = s / GRID
    uu = u / s[:, None]
    B = d[:, 0] / s
    A = (C[:, 0] + 25.0 * d[:, 0]) / s - (uu * js[None, :]).sum(1)

    prm = np.zeros((N_CH, 85), _f64)
    # prelu leaves: u*max(t,j) = u*j + stream - a*beta*(t-j),
    # stream = prelu(beta*(t-j); a), beta = 1+|u|, a = 1 - u/beta
    for i, k in enumerate(P_KS):
        j = js[k]
        beta = 1.0 + np.abs(uu[:, k])
        a = 1.0 - uu[:, k] / beta
        A += uu[:, k] * j + a * beta * j
        B += -a * beta
        prm[:, 52 + i] = beta * alpha
        prm[:, 63 + i] = -beta * j
        prm[:, 74 + i] = a
    prm[:, 0] = alpha
    prm[:, 1] = B * alpha
    prm[:, 2] = A
    prm[:, 3:52] = uu
    return prm.astype(_f32)


def _register_ops():
    import concourse.dve_ops as dve_ops
    from concourse.dve_spec import Spec, Src0, Src1, C0, C1, C2, One, lower, maxx
    from concourse.dve_uop import DveOpSpec

    def reg(name, spec, rd1):
        for op in dve_ops.OPS:
            if op.name == name:
                return op
        row = max(dve_ops._SUB_OPCODE_FOR_NAME.values()) + 1
        assert row < 0x20
        dve_ops._SUB_OPCODE_FOR_NAME[name] = row
        uops = lower(spec, ver="v3")
        sha = DveOpSpec(name=name, opcode=row, uops=uops, rd1_en=rd1).sha("v3")
        op = dve_ops.DveOp(name, spec, subdim=False, uops_sha={"v3": sha})
        dve_ops.OPS.append(op)
        dve_ops.CUSTOM_DVE_SPECS[name] = spec
        return op

    # acc' = acc + C0*max(t, j) + C1*max(t, j+1);  j = imm2, j+1 hoisted.
    maxpair = reg("LS_MAXPAIR", Spec(body=(
        C0 * maxx(Src0, C2) + C1 * maxx(Src0, C2 + One)) + Src1), True)
    # chain seed: same two-knot form without the accumulator stream
    maxpair0 = reg("LS_MAXPAIR0", Spec(body=(
        C0 * maxx(Src0, C2) + C1 * maxx(Src0, C2 + One))), False)
    return maxpair, maxpair0


def _build():
    if "nc" in _built:
        return _built["nc"]
    import concourse.bass as bass
    import concourse.mybir as mybir
    from concourse.library_overlay import lower_extended_insts

    MAXPAIR, MAXPAIR0 = _register_ops()
    F32 = mybir.dt.float32
    Ident = mybir.ActivationFunctionType.Identity
    Prelu = mybir.ActivationFunctionType.Prelu
    Alu = mybir.AluOpType

    nc = bass.Bass()
    x_in = nc.declare_dram_parameter("x", [GROUPS, 128, FREE], F32, isOutput=False)
    prm = nc.declare_dram_parameter("prm", [128, 85], F32, isOutput=False)
    y_out = nc.declare_dram_parameter("y", [GROUPS, 128, FREE], F32, isOutput=True)

    xb = [nc.alloc_sbuf_tensor(f"xb{i}", [128, F], F32).ap() for i in range(2)]
    tb = [nc.alloc_sbuf_tensor(f"tb{i}", [128, F], F32).ap() for i in range(2)]
    dl = [[nc.alloc_sbuf_tensor(f"dl{p}{i}", [128, F], F32).ap() for i in range(2)]
          for p in range(2)]
    gl = [[nc.alloc_sbuf_tensor(f"gl{p}{i}", [128, F], F32).ap() for i in range(2)]
          for p in range(2)]
    rr = [nc.alloc_sbuf_tensor(f"rr{i}", [128, F], F32).ap() for i in range(RING)]
    rc = [[nc.alloc_sbuf_tensor(f"rc{p}{m}", [128, F], F32).ap() for m in range(N_C)]
          for p in range(2)]
    pb = nc.alloc_sbuf_tensor("pb", [128, 85], F32).ap()

    a_s = pb[:, 0:1]          # alpha
    i_sc = pb[:, 1:2]         # B*alpha
    i_bi = pb[:, 2:3]         # A

    def u_ap(k):              # u_k column
        return pb[:, 3 + k:4 + k]

    def unit_slice(u):
        g, ci = divmod(u, FREE // F)
        return g, ci * F

    with (nc.Block() as block,
          nc.semaphore("s_in") as s_in,
          nc.semaphore("s_act") as s_act,
          nc.semaphore("s_dve") as s_dve,
          nc.semaphore("s_gp") as s_gp,
          nc.semaphore("s_o1") as s_o1,
          nc.semaphore("s_o2") as s_o2):

        @block.sync
        def _(sync):
            sync.dma_start(out=pb[:], in_=prm[:]).then_inc(s_in, 16)

            def dma_in(u):
                if u >= 2:
                    sync.wait_ge(s_act, N_ACT_OPS * (u - 1))
                g, off = unit_slice(u)
                sync.dma_start(out=xb[u % 2][:],
                               in_=x_in[g, :, off:off + F]).then_inc(s_in, 16)

            dma_in(0)
            dma_in(1)
            for u in range(UNITS):
                # plain store of the GPSIMD partial (gl final in [0])
                sync.wait_ge(s_gp, N_P * (u + 1))
                g, off = unit_slice(u)
                sync.dma_start(out=y_out[g, :, off:off + F],
                               in_=gl[u % 2][0][:]).then_inc(s_o1, 16)
                if u + 2 < UNITS:
                    dma_in(u + 2)

        @block.scalar
        def _(scalar):
            for u in range(UNITS):
                scalar.wait_ge(s_in, 16 + 16 * (u + 1))
                # t = alpha * x   (tb last read by DVE(u-2))
                if u >= 2:
                    scalar.wait_ge(s_dve, N_DVE * (u - 1))
                scalar.activation(out=tb[u % 2][:], in_=xb[u % 2][:],
                                  func=Ident, scale=a_s,
                                  bias=0.0).then_inc(s_act, 1)
                # acc0 = A + B*t -> gl[p][1]  (last read by GP(u-2) join_10)
                if u >= 2:
                    scalar.wait_ge(s_gp, N_P * (u - 1))
                scalar.activation(out=gl[u % 2][1][:], in_=xb[u % 2][:],
                                  func=Ident, scale=i_sc,
                                  bias=i_bi).then_inc(s_act, 1)
                # signed prelu leaves: N_P for GPSIMD joins, N_C for DMA merge
                for i in range(N_P):
                    if u >= 1:
                        # slot i reused each unit: wait join_i of unit u-1
                        scalar.wait_ge(s_gp, N_P * (u - 1) + i + 1)
                    scalar.activation(out=rr[i][:], in_=xb[u % 2][:],
                                      func=Prelu,
                                      scale=pb[:, 52 + i:53 + i],
                                      bias=pb[:, 63 + i:64 + i],
                                      alpha=pb[:, 74 + i:75 + i]).then_inc(s_act, 1)
                for m in range(N_C):
                    i = N_P + m
                    if u >= 2:
                        # rc slot reused after chain(u-2)'s accum stores done
                        scalar.wait_ge(s_o2, 16 * (1 + N_C) * (u - 1))
                    scalar.activation(out=rc[u % 2][m][:], in_=xb[u % 2][:],
                                      func=Prelu,
                                      scale=pb[:, 52 + i:53 + i],
                                      bias=pb[:, 63 + i:64 + i],
                                      alpha=pb[:, 74 + i:75 + i]).then_inc(s_act, 1)

        @block.vector
        def _(vector):
            for u in range(UNITS):
                vector.wait_ge(s_act, N_ACT_OPS * u + 1)
                if u >= 2:
                    # dl tiles freed once the accum stores of u-2 completed
                    vector.wait_ge(s_o2, 16 * (1 + N_C) * (u - 1))
                for i in range(N_DVE):
                    k1 = DVE_KS[2 * i]
                    j1 = float(k1 - 24)
                    if i == 0:
                        vector._custom_dve(
                            MAXPAIR0,
                            out=dl[u % 2][0][:],
                            in0=tb[u % 2][:],
                            s0=u_ap(k1), s1=u_ap(k1 + 1),
                            imm2=j1).then_inc(s_dve, 1)
                    else:
                        vector._custom_dve(
                            MAXPAIR,
                            out=dl[u % 2][i % 2][:],
                            in0=tb[u % 2][:],
                            in1=dl[u % 2][(i + 1) % 2][:],
                            s0=u_ap(k1), s1=u_ap(k1 + 1),
                            imm2=j1).then_inc(s_dve, 1)

        @block.gpsimd
        def _(gp):
            def dma2(u):
                # accumulating stores: DVE partial (dl final in [0]) plus the
                # N_C leaf tiles; ordered after the plain store of the same
                # unit via s_o1, and against each other by Pool-queue FIFO.
                gp.wait_ge(s_dve, N_DVE * (u + 1))
                gp.wait_ge(s_o1, 16 * (u + 1))
                g, off = unit_slice(u)
                gp.dma_start(out=y_out[g, :, off:off + F],
                             in_=dl[u % 2][0][:],
                             accum_op=Alu.add).then_inc(s_o2, 16)
                for m in range(N_C):
                    gp.dma_start(out=y_out[g, :, off:off + F],
                                 in_=rc[u % 2][m][:],
                                 accum_op=Alu.add).then_inc(s_o2, 16)

            for u in range(UNITS):
                if u >= 2:
                    dma2(u - 2)
                for i in range(N_P):
                    gp.wait_ge(s_act, N_ACT_OPS * u + 3 + i)
                    src = gl[u % 2][1][:] if i % 2 == 0 else gl[u % 2][0][:]
                    dst = gl[u % 2][0][:] if i % 2 == 0 else gl[u % 2][1][:]
                    gp.tensor_tensor(
                        out=dst,
                        in0=rr[i][:],
                        in1=src,
                        op=Alu.add).then_inc(s_gp, 1)
            dma2(UNITS - 2)
            dma2(UNITS - 1)

    lower_extended_insts(nc)
    _built["nc"] = nc
    return nc


def kernel(x, coefficients_vect, scaling_coeffs_vect):
    from concourse.bass_utils import run_bass_kernel_spmd
    from concourse import bass2jax
    bass2jax.install_neuronx_cc_hook()

    x = np.ascontiguousarray(np.asarray(x, _f32))
    coeff = np.asarray(coefficients_vect, _f32).reshape(-1)
    scal = np.asarray(scaling_coeffs_vect, _f32).reshape(-1)

    prm_ch = _host_params(coeff, scal)                 # [64, 85]
    prm_full = np.ascontiguousarray(np.tile(prm_ch, (2, 1)))  # [128, 85]

    nb = N_BATCH // NCORES
    in_maps = []
    for i in range(NCORES):
        xi = x[nb * i:nb * (i + 1)].reshape(GROUPS, 128, FREE)
        in_maps.append({"x": np.ascontiguousarray(xi), "prm": prm_full})

    nc = _build()
    import os
    trace = bool(os.environ.get("LS_TRACE"))
    res = run_bass_kernel_spmd(nc, in_maps, list(range(NCORES)), trace=trace)
    if trace:
        print("exec_time_ns:", res.exec_time_ns)

    out = np.empty((N_BATCH, N_CH, H, W), _f32)
    for i in range(NCORES):
        out[nb * i:nb * (i + 1)] = np.asarray(res.results[i]["y"]).reshape(
            nb, N_CH, H, W)
    return out
